# revision 35
# baseline (speedup 1.0000x reference)
"""Causal self-attention Trainium2 kernel.

Full inputs in, full output out. Internally: 8 NeuronCores, data-parallel on
batch (2) x tensor-parallel on heads (4 groups of 4 heads). Each core computes
its 4 heads' attention for its batch in a transposed layout (head-dim /
key-dim on partitions) and a partial output projection; the host sums the 4
partial projections per batch and adds b_proj.

Per-core device program:
  kqv^T = W8.T @ x8^T (+8b)            fp8 DoubleRow matmuls, [768, 2048]
                                        (x, 8W in e4m3; result = 8*(xW+b))
  per head: S^T = k^T.T-block @ q^T     [128m x 512n] bf16 blocks, causal-trim
            P^T = exp(S^T / 512)        (1/512 = 1/(8*8*sqrt(DH)) act scale)
            U^T = [v|1].T-block @ P^T   rows 0-63 = 8*unnorm sa^T, row 64 = denom
            sa^T = U^T[0:64] * (1/denom broadcast)
  partial out^T = (WprojT/8).T @ sa^T   [1024, 2048] -> bf16 DRAM

Schedule: attention starts as soon as kqv chunk 0 lands; later kqv groups,
v-transposes, pair-1 kqv, and projections all run as PE filler inside the
ScalarE-exp-bound attention windows. Each chunk's normalization is deferred
into the next chunk's filler stream (the PE denominator-broadcast then never
waits at a chunk tail).
"""
import sys, os
sys.path.insert(0, '/opt/trn_rl_repo')
os.environ.setdefault("JAX_PLATFORMS", "")

import numpy as np
import ml_dtypes

import concourse.bass as bass
import concourse.bacc as bacc
import concourse.tile as tile
import concourse.mybir as mybir
from concourse import bass_utils

B, N, D, H, DH = 2, 2048, 1024, 16, 64
G = 4              # heads per core
NCORES = 8
NCH = 512          # n-chunk width
NJ = N // NCH      # 4 n-chunks
NMB = N // 128     # 16 m-blocks
EW = G * 3 * DH    # 768 packed kqv width per core
bf16 = ml_dtypes.bfloat16
fp8 = ml_dtypes.float8_e4m3
f32 = np.float32
AF = mybir.ActivationFunctionType
DR = mybir.MatmulPerfMode.DoubleRow
EXP_SCALE = 1.0 / 512.0   # undo the 8x on k and q, then 1/sqrt(DH)

_cache = {}


def _build_program():
    nc = bacc.Bacc("TRN2", target_bir_lowering=False, debug=False, num_devices=NCORES)

    xq_d = nc.dram_tensor("xq", [NJ, 128, 8, NCH], mybir.dt.float8e4, kind="ExternalInput").ap()
    xt_d = nc.dram_tensor("xt", [NJ, 128, 8, NCH], mybir.dt.bfloat16, kind="ExternalInput").ap()
    w8_d = nc.dram_tensor("w8", [128, 8, 512], mybir.dt.float8e4, kind="ExternalInput").ap()
    wv_d = nc.dram_tensor("wv", [128, 8 * 256 + 8], mybir.dt.bfloat16, kind="ExternalInput").ap()
    wpt_d = nc.dram_tensor("wpt", [2 * 128, D], mybir.dt.bfloat16, kind="ExternalInput").ap()
    out_d = nc.dram_tensor("outt", [D, N], mybir.dt.bfloat16, kind="ExternalOutput").ap()
    dbg = None
    if os.environ.get("KDBG") == "1":
        dbg = {
            "sa": nc.dram_tensor("dbg_sa", [2, 128, N], mybir.dt.bfloat16, kind="ExternalOutput").ap(),
            "kqvT": nc.dram_tensor("dbg_kqvT", [6, 128, N], mybir.dt.bfloat16, kind="ExternalOutput").ap(),
            "vp": nc.dram_tensor("dbg_vp", [4, 128, 16 * 66], mybir.dt.bfloat16, kind="ExternalOutput").ap(),
        }

    with tile.TileContext(nc) as tc:
        _emit(nc, tc, xq_d, xt_d, w8_d, wv_d, wpt_d, out_d, dbg)

    nc.compile()
    return nc


def _emit(nc, tc, xq_d, xt_d, w8_d, wv_d, wpt_d, out_d, dbg=None):
    from contextlib import ExitStack

    dt = mybir.dt
    ctx = ExitStack()
    with ctx:
        consts = ctx.enter_context(tc.tile_pool(name="consts", bufs=1))
        work = ctx.enter_context(tc.tile_pool(name="work", bufs=1))

        # ---- input loads. w8 first, then the x windows in consumption
        # order: the first kqv group only needs w8 + window 0 (~1.25MB), so
        # the PE starts ~5us in instead of waiting for the full input set.
        # DMA lines dispatch FIFO in emission order, sharded uniformly
        # across the 16 HW queues (~245GB/s aggregate) — so order transfers
        # by first use and keep the dma_start count low (each costs ~250ns
        # of SP issue time before any line moves).
        w8_sb = consts.tile([128, 8, 512], dt.float8e4, name="w8", tag="w8")
        xq_sb = consts.tile([128, NJ, 8, NCH], dt.float8e4, name="xq", tag="xq")
        wvb_sb = consts.tile([128, 8 * 256 + 12], dt.bfloat16, name="wvb", tag="wvb")
        wv_sb = wvb_sb[:, 0:2048].rearrange("p (s c) -> p s c", s=8)
        bias_f32 = wvb_sb[:, 2048:2060].bitcast(dt.float32)
        b_sb = [bias_f32[:, i:i + 1] for i in range(6)]
        xt_sb = consts.tile([128, NJ, 8, NCH], dt.bfloat16, name="xt", tag="xt")
        wpt_sb = [consts.tile([128, D], dt.bfloat16, name=f"wpt{kc}", tag=f"wpt{kc}")
                  for kc in range(2)]

        nc.sync.dma_start(wvb_sb[:, 2048:2060], wv_d[:, 2048:2060])  # bias
        nc.sync.dma_start(w8_sb[:], w8_d[:])
        nc.sync.dma_start(xq_sb[:, 0], xq_d[0])
        nc.sync.dma_start(wvb_sb[:, 0:2048], wv_d[:, 0:2048])
        nc.sync.dma_start(xt_sb[:, 0], xt_d[0])
        nc.sync.dma_start(xq_sb[:, 1], xq_d[1])
        nc.sync.dma_start(xt_sb[:, 1], xt_d[1])
        nc.sync.dma_start(xq_sb[:, 2], xq_d[2])
        nc.sync.dma_start(xt_sb[:, 2], xt_d[2])
        nc.sync.dma_start(xq_sb[:, 3], xq_d[3])
        nc.sync.dma_start(wpt_sb[0][:], wpt_d[0:128, :])
        nc.sync.dma_start(wpt_sb[1][:], wpt_d[128:256, :])
        nc.sync.dma_start(xt_sb[:, 3], xt_d[3])
        # ones row + identity generated on-device (kills two tiny DMAs)
        ones_sb = consts.tile([128, 64], dt.bfloat16, name="ones", tag="ones")
        nc.gpsimd.memset(ones_sb[:], 1.0)
        # warm the ScalarE act table (1.3us load) while input DMAs stream,
        # so the first real exp doesn't pay it on the critical path
        warm = consts.tile([1, 2], dt.float32, name="warm", tag="warm")
        nc.vector.memset(warm[:], 1.0)
        nc.scalar.activation(warm[:], warm[:], AF.Exp)
        ident = consts.tile([128, 128], dt.bfloat16, name="ident", tag="ident")
        nc.gpsimd.memset(ident[:], 1.0)
        nc.gpsimd.affine_select(
            out=ident[:], in_=ident[:], compare_op=mybir.AluOpType.is_ge,
            fill=0.0, base=0, pattern=[[1, 128]], channel_multiplier=-1)
        nc.gpsimd.affine_select(
            out=ident[:], in_=ident[:], compare_op=mybir.AluOpType.is_ge,
            fill=0.0, base=0, pattern=[[-1, 128]], channel_multiplier=1)
        # on-device causal mask (local coords: keep col>=row); one tile
        # serves every diagonal block via offset slicing
        cmask = consts.tile([128, NCH], dt.bfloat16, name="cmask", tag="cmask")
        nc.gpsimd.memset(cmask[:], 1.0)
        nc.gpsimd.affine_select(
            out=cmask[:], in_=cmask[:],
            compare_op=mybir.AluOpType.is_ge, fill=0.0,
            base=0, pattern=[[1, NCH]], channel_multiplier=-1,
        )

        # persistent kqv^T, v', sa^T buffers
        kqvT = [work.tile([128, N], dt.bfloat16, name=f"kqvT{i}", tag=f"kqvT{i}")
                for i in range(6)]
        vp = [work.tile([128, NMB, 66], dt.bfloat16, name=f"vp{h}", tag=f"vp{h}")
              for h in range(G)]
        saT = [work.tile([128, N], dt.bfloat16, name=f"saT{kc}", tag=f"saT{kc}")
               for kc in range(2)]

        # per-head slices (pair packing [k_e|k_o|q_e|q_o|v_e|v_o])
        def head_slices(h):
            p, o = h // 2, (h % 2) * 64
            kT = kqvT[3 * p][o:o + 64, :]
            qT = kqvT[3 * p + 1][o:o + 64, :]
            vT = kqvT[3 * p + 2][o:o + 64, :]
            return kT, qT, vT, o

        # 8 PSUM banks: ps(4, S tiles / denom-broadcasts / v-transposes)
        # + pu(2, attention accumulators) + pp(2, kqv groups & projections)
        ps = ctx.enter_context(tc.tile_pool(name="ps", bufs=4, space="PSUM"))
        pu = ctx.enter_context(tc.tile_pool(name="pu", bufs=2, space="PSUM"))
        pp = ctx.enter_context(tc.tile_pool(name="pp", bufs=2, space="PSUM"))
        pPool = ctx.enter_context(tc.tile_pool(name="pP", bufs=8))
        paux = ctx.enter_context(tc.tile_pool(name="paux", bufs=2))
        pout = ctx.enter_context(tc.tile_pool(name="pout", bufs=3))

        KQ_DST = (0, 1, 3, 4)   # kq group g -> kqvT index (pair k, q rows)
        V_DST = (2, 5)          # v group vc -> kqvT index

        def emit_kq_group(g, jj, bias_on_dve):
            # fp8 DoubleRow: two 128-row K-subtiles per matmul
            mc = KQ_DST[g]
            ps_t = pp.tile([128, NCH], dt.float32, tag="pp", name="kqvp")
            for s in range(0, 8, 2):
                nc.tensor.matmul(
                    ps_t[:],
                    w8_sb[:, s:s + 2, g * 128:(g + 1) * 128],
                    xq_sb[:, jj, s:s + 2, :],
                    start=(s == 0), stop=(s == 6),
                    perf_mode=DR,
                )
            dst = kqvT[mc][:, jj * NCH:(jj + 1) * NCH]
            if bias_on_dve:
                nc.vector.tensor_scalar_add(dst, ps_t[:], b_sb[mc][:])
            else:
                nc.scalar.activation(dst, ps_t[:], AF.Identity, bias=b_sb[mc][:])

        def emit_v_group(vc, jj, bias_on_dve):
            # v stays bf16: its quantization error passes straight to the
            # output (fp8 v alone costs 3.6e-2 absmax rel — over the gate)
            mc = V_DST[vc]
            ps_t = pp.tile([128, NCH], dt.float32, tag="pp", name="kqvp")
            for s in range(8):
                nc.tensor.matmul(
                    ps_t[:],
                    wv_sb[:, s, vc * 128:(vc + 1) * 128],
                    xt_sb[:, jj, s, :],
                    start=(s == 0), stop=(s == 7),
                )
            dst = kqvT[mc][:, jj * NCH:(jj + 1) * NCH]
            if bias_on_dve:
                nc.vector.tensor_scalar_add(dst, ps_t[:], b_sb[mc][:])
            else:
                nc.scalar.activation(dst, ps_t[:], AF.Identity, bias=b_sb[mc][:])

        def emit_vp_quad(h, q):
            # transpose 4 v-blocks into one psum tile, one strided copy out
            _, _, vT_h, o = head_slices(h)
            tp = ps.tile([128, 256], dt.bfloat16, tag="s", name="vtp")
            for i in range(4):
                mb = 4 * q + i
                nc.tensor.matmul(
                    tp[:, 64 * i:64 * (i + 1)],
                    vT_h[:, mb * 128:(mb + 1) * 128],
                    ident[o:o + 64, o:o + 64],
                    is_transpose=True, skip_group_check=True,
                )
            nc.vector.tensor_copy(vp[h][:, 4 * q:4 * q + 4, 0:64], tp[:])

        def emit_proj_oc(j, oc, on_scalar=False):
            # psum drain on ScalarE (slack in phase C, idle in the tail);
            # bf16 partials halve the output DMA, host sums in fp32
            nsl = slice(j * NCH, (j + 1) * NCH)
            pp_t = pp.tile([128, NCH], dt.float32, tag="pp")
            for kc in range(2):
                nc.tensor.matmul(
                    pp_t[:],
                    wpt_sb[kc][:, oc * 128:(oc + 1) * 128],
                    saT[kc][:, nsl],
                    start=(kc == 0), stop=(kc == 1),
                )
            o_t = pout.tile([128, NCH], dt.bfloat16, tag="o")
            if on_scalar:
                nc.scalar.copy(o_t[:], pp_t[:])
            else:
                nc.vector.tensor_copy(o_t[:], pp_t[:])
            nc.sync.dma_start(out_d[oc * 128:(oc + 1) * 128, nsl], o_t[:])

        def emit_attn_chunk(j, p, fillers, tail_norm=False):
            """Attention for chunk j, head pair p, with `fillers` (zero-arg
            emitters of dependency-free PE work) spread across the m-loop.
            Returns the chunk's deferred normalization steps as filler
            closures for the NEXT chunk."""
            nsl = slice(j * NCH, (j + 1) * NCH)
            nm = 4 * (j + 1)
            pair = (2 * p, 2 * p + 1)
            u_t = {h: pu.tile([65, NCH], dt.float32, tag="u", name=f"u{h}")
                   for h in pair}
            p_tiles = {h: [None] * nm for h in pair}
            offs = [0] * nm
            fill = list(fillers)
            per_step = max(1, -(-len(fill) // max(nm, 1)))

            def emit_s(h, mi):
                kT, qT, _, _ = head_slices(h)
                r = mi - 4 * j
                off = 128 * r if r > 0 else 0
                offs[mi] = off
                s_t = ps.tile([128, NCH], dt.float32, tag="s")
                nc.tensor.matmul(
                    s_t[:, off:],
                    kT[:, mi * 128:(mi + 1) * 128],
                    qT[:, j * NCH + off:(j + 1) * NCH],
                    start=True, stop=True,
                )
                p_t = pPool.tile([128, NCH], dt.bfloat16, tag="p")
                if r >= 0:
                    e_t = pPool.tile([128, NCH], dt.bfloat16, tag="e")
                    nc.scalar.activation(e_t[:, off:], s_t[:, off:], AF.Exp,
                                         scale=EXP_SCALE)
                    if j == 0:
                        nc.vector.tensor_mul(
                            p_t[:, off:], e_t[:, off:], cmask[:, 0:NCH - off])
                    else:
                        nc.gpsimd.affine_select(
                            out=p_t[:, off:], in_=e_t[:, off:],
                            compare_op=mybir.AluOpType.is_ge, fill=0.0,
                            base=0, pattern=[[1, NCH - off]],
                            channel_multiplier=-1,
                        )
                else:
                    nc.scalar.activation(p_t[:, off:], s_t[:, off:], AF.Exp,
                                         scale=EXP_SCALE)
                p_tiles[h][mi] = p_t

            def emit_pv(h, mi):
                off = offs[mi]
                nc.tensor.matmul(
                    u_t[h][:, off:],
                    vp[h][:, mi, 0:65],
                    p_tiles[h][mi][:, off:],
                    start=(mi == 0), stop=(mi == nm - 1),
                    skip_group_check=True,
                )

            depth = 2
            for mi in range(nm):
                for n_ in range(per_step):
                    if fill:
                        fill.pop(0)()
                for h in pair:
                    emit_s(h, mi)
                if mi >= depth:
                    for h in pair:
                        emit_pv(h, mi - depth)
            for mi in range(max(nm - depth, 0), nm):
                for h in pair:
                    emit_pv(h, mi)
            while fill:
                fill.pop(0)()

            # copy the raw denominator rows out now; defer the PE broadcast
            # + reciprocal + multiply into the next chunk's filler stream.
            # Odd head first: its result needs an extra partition-shift DMA,
            # so starting it first lets that DMA overlap the even head.
            norm_steps = []
            for h in (pair[1], pair[0]):
                if True:
                    dr_t = paux.tile([65, NCH], dt.bfloat16, tag="dr")
                    if tail_norm:
                        nc.scalar.copy(dr_t[64:65, :], u_t[h][64:65, :])
                    else:
                        nc.vector.tensor_copy(dr_t[64:65, :], u_t[h][64:65, :])
                else:
                    # tail: 1/d = exp(-ln d) on ScalarE (idle once the last
                    # exp retires), reading the PSUM row directly
                    lnr = paux.tile([65, NCH], dt.float32, tag="lnr")
                    nc.scalar.activation(lnr[64:65, :], u_t[h][64:65, :], AF.Ln)

                def norm(h=h, u_h=u_t[h], nsl=nsl, dr_t=dr_t):
                    bcp = ps.tile([128, NCH], dt.float32, tag="s", name=f"bcp{h}")
                    nc.tensor.matmul(bcp[0:64, :], ones_sb[64:65, 0:64],
                                     dr_t[64:65, :], start=True, stop=True)
                    bc = paux.tile([64, NCH], dt.float32, tag="bc")
                    nc.vector.tensor_copy(bc[:], bcp[0:64, :])
                    rc = paux.tile([64, NCH], dt.float32, tag="rc64")
                    nc.vector.reciprocal_approx_fast(rc[:], bc[:])
                    kc, row = h // 2, (h % 2) * 64
                    if row == 0:
                        nc.vector.tensor_mul(saT[kc][0:64, nsl],
                                             u_h[0:64, :], rc[:])
                    else:
                        tmp = paux.tile([64, NCH], dt.bfloat16, tag="tmp")
                        nc.vector.tensor_mul(tmp[:], u_h[0:64, :], rc[:])
                        nc.sync.dma_start(saT[kc][64:128, nsl], tmp[:])
                norm_steps.append(norm)
            return norm_steps

        # A: minimal prologue — kqv chunk 0 for pair 0 + the first v-quad,
        # so attention chunk 0 can start while the rest of x still streams in
        for h in range(G):
            nc.gpsimd.memset(vp[h][:, :, 64:65], 1.0)
        emit_kq_group(0, 0, bias_on_dve=True)
        emit_kq_group(1, 0, bias_on_dve=True)
        emit_v_group(0, 0, bias_on_dve=True)
        emit_vp_quad(0, 0)
        emit_vp_quad(1, 0)

        # B: attention pair 0; fillers bring in the remaining pair-0 kqv
        # chunks + v-quads just ahead of the chunk that consumes them, then
        # pair 1's kqv. Each chunk's normalization defers into the next.
        b_fill = {
            0: [('kq', 0, 1), ('kq', 1, 1), ('v', 0, 1),
                ('q', 0, 1), ('q', 1, 1)],
            1: [('kq', 0, 2), ('kq', 1, 2), ('v', 0, 2),
                ('q', 0, 2), ('q', 1, 2), ('kq', 2, 0)],
            2: [('kq', 0, 3), ('kq', 1, 3), ('v', 0, 3),
                ('q', 0, 3), ('q', 1, 3), ('kq', 3, 0), ('v', 1, 0),
                ('kq', 2, 1)],
            3: [('kq', 3, 1), ('v', 1, 1), ('kq', 2, 2), ('kq', 3, 2),
                ('v', 1, 2), ('kq', 2, 3), ('kq', 3, 3), ('v', 1, 3)],
        }
        pending_norm = []
        for j in range(NJ):
            fillers = list(pending_norm)
            for it in b_fill[j]:
                if it[0] == 'q':
                    fillers.append(lambda h=it[1], q=it[2]: emit_vp_quad(h, q))
                elif it[0] == 'kq':
                    fillers.append(lambda g=it[1], jj=it[2]:
                                   emit_kq_group(g, jj, bias_on_dve=True))
                else:
                    fillers.append(lambda vc=it[1], jj=it[2]:
                                   emit_v_group(vc, jj, bias_on_dve=True))
            pending_norm = emit_attn_chunk(j, 0, fillers)

        # C: attention pair 1 in order (1, 2, 3, 0): the two big chunks
        # absorb the finished chunks' projections as PE filler, and the
        # end-of-kernel tail (last normalization + projection + output DMA)
        # hangs off the SMALLEST chunk.
        c_order = (1, 2, 3, 0)
        quads_at = {1: (0, 1), 2: (2,), 3: (3,), 0: ()}
        prev_j = None
        for j in c_order:
            fillers = []
            if j == 1:
                # quad 0 feeds this chunk's first PV (emitted at m-step 2):
                # it must lead the filler stream or the PE queue would wait
                # on transposes emitted behind it
                fillers += [lambda: emit_vp_quad(2, 0), lambda: emit_vp_quad(3, 0)]
            fillers += list(pending_norm)
            for q in quads_at[j]:
                if j == 1 and q == 0:
                    continue
                fillers += [lambda q=q: emit_vp_quad(2, q),
                            lambda q=q: emit_vp_quad(3, q)]
            if prev_j is not None:
                fillers += [(lambda oc=oc, jj=prev_j:
                             emit_proj_oc(jj, oc, on_scalar=(oc % 2 == 0)))
                            for oc in range(8)]
            pending_norm = emit_attn_chunk(j, 1, fillers)
            prev_j = j
        for step in pending_norm:
            step()
        for oc in range(8):
            emit_proj_oc(0, oc, on_scalar=(oc % 2 == 0))

        if dbg is not None:
            for kc in range(2):
                nc.sync.dma_start(dbg["sa"][kc], saT[kc][:])
            for i in range(6):
                nc.sync.dma_start(dbg["kqvT"][i], kqvT[i][:])
            for h in range(4):
                nc.sync.dma_start(dbg["vp"][h], vp[h].rearrange("p a b -> p (a b)"))


def _host_prep(x, W_kqv, b_kqv, W_proj):
    """Build the 8 per-core input maps."""
    x = np.asarray(x, dtype=f32)
    W_kqv = np.asarray(W_kqv, dtype=f32)
    b_kqv = np.asarray(b_kqv, dtype=f32)
    W_proj = np.asarray(W_proj, dtype=f32)

    in_maps = []
    for c in range(NCORES):
        b, g = c // 4, c % 4
        heads = [4 * g + i for i in range(4)]
        # pack per pair: [k_e | k_o | q_e | q_o | v_e | v_o], all scaled 8x
        # (fp8-friendly weight range; the 8x on k/q is undone by the exp
        # scale, the 8x on v by the 1/8 folded into W_proj)
        kqcols, vcols, bcols = [], [], []
        for p in range(2):
            he, ho = heads[2 * p], heads[2 * p + 1]
            for sec in range(3):  # k, q, v
                scl = 8.0 if sec < 2 else 1.0
                for h in (he, ho):
                    wsec = W_kqv[h][:, sec * 64:(sec + 1) * 64] * scl
                    (kqcols if sec < 2 else vcols).append(wsec)
                    bcols.append(b_kqv[h][sec * 64:(sec + 1) * 64] * scl)
        wkq = np.concatenate(kqcols, axis=1)             # [1024, 512]
        wv = np.concatenate(vcols, axis=1)               # [1024, 256]
        bpack = np.concatenate(bcols).astype(f32)        # [768]
        xT = np.ascontiguousarray(x[b].T)                # [1024, 2048]
        # [K=s*128+p, n] -> [jj, p, s, ncol]
        xre = xT.reshape(8, 128, NJ, NCH).transpose(2, 1, 0, 3)
        wv_flat = wv.reshape(8, 128, 256).transpose(1, 0, 2).reshape(128, 2048)
        wvb = np.zeros((128, 8 * 256 + 8), dtype=bf16)
        wvb[:, 0:2048] = wv_flat.astype(bf16)
        wvb[:, 2048:2054] = bpack.reshape(6, 128).T.astype(bf16)
        in_maps.append({
            "xq": np.ascontiguousarray(xre).astype(fp8),
            "xt": np.ascontiguousarray(xre).astype(bf16),
            "w8": np.ascontiguousarray(
                wkq.reshape(8, 128, 512).transpose(1, 0, 2)).astype(fp8),
            "wv": wvb,
            "wpt": np.ascontiguousarray(
                W_proj[:, 256 * g:256 * (g + 1)].T).astype(bf16),
        })
    return in_maps


def run(x, W_kqv, b_kqv, W_proj, b_proj, trace=False, trace_cores=None):
    if "nc" not in _cache:
        _cache["nc"] = _build_program()
    nc = _cache["nc"]
    in_maps = _host_prep(x, W_kqv, b_kqv, W_proj)
    res = bass_utils.run_bass_kernel_spmd(
        nc, in_maps, core_ids=list(range(NCORES)),
        trace=trace, trace_cores=trace_cores,
    )
    b_proj = np.asarray(b_proj, dtype=f32)
    out = np.zeros((B, N, D), dtype=f32)
    for b in range(B):
        acc = res.results[4 * b]["outt"].astype(f32)
        for g in range(1, 4):
            acc = acc + res.results[4 * b + g]["outt"].astype(f32)
        out[b] = acc.T + b_proj[None, :]
    return out, res


def kernel(x, W_kqv, b_kqv, W_proj, b_proj):
    out, _ = run(x, W_kqv, b_kqv, W_proj, b_proj, trace=False)
    return out
